# revision 47
# baseline (speedup 1.0000x reference)
"""ALiBi causal attention on 8 Trainium2 NeuronCores.

Sharding: tensor-parallel over heads (2 heads/core). Core c owns global
heads c (steep slope) and 8+c (shallow slope) so that ALiBi block-skipping
gives every core the same instruction stream: steep heads only attend to
the ~8 k-blocks nearest the diagonal (older blocks decay below e^-30 of
the max weight), shallow heads attend to everything. Two batch-split
AllToAlls redistribute the (normalized, transposed) attention outputs so
each core owns 256 tokens of each batch for the output projection; the
b0 AllToAll overlaps b1's attention compute.

Layout choices (all chosen to avoid on-chip transposes):
  - x is passed host-transposed as xT [D=1024, B*T=4096] in bf16.
  - Q/K are produced in "head-transposed" layout [head_dim, tokens] with
    THREE augmented contraction rows carrying the ALiBi bias through the
    score matmul exactly:
      row 64: K=1,      Q=-8*slope*bf16(i)   (per-query term; any rounding
              cancels in softmax, so bf16 is safe)
      row 65: K=kb,     Q=C   where C = bf16(1024*slope); kb<=15 is exact
              in bf16 so C*kb accumulates exactly in f32
      row 66: K=kb,     Q=Dr  where Dr = bf16(1024*slope - C) mops up the
              rounding of C (double-bf16 trick)
    leaving only slope*p (p = partition index, exact f32) for the ScalarE
    exp bias -- which is then the same for every k-block of a head, so
    one exp instruction spans a PAIR of k-blocks (halves ACT op count;
    ACT exp throughput is the attention-phase bottleneck).
  - Scores are computed transposed: ST[k, q] = K'.T-block @ Q', so the
    softmax reduction (over k) aligns with the AV matmul contraction and
    the denominator falls out of a ones-column appended to V.
  - Causal masking: only the diagonal-intersecting k-block per q-tile
    needs a 128x128 triangular min-clamp; fully-masked columns are never
    computed or streamed.

Tiles are deliberately small/chunked (xT per [k,512-token] block, Q/K per
[head, 512-token] chunk, V per [token-block]) because Tile's dependency
tracking is per-tile: projection chunk i feeds attention group i 1:1 in
the schedule. DMA queues: weights + xT stream on the sync queue and
constants/aug rows on the GpSimd queue -- never on a compute engine's
queue, where a backed-up HWDGE ring would stall that engine's compute.
"""

import sys

if "/opt/trn_rl_repo" not in sys.path:
    sys.path.insert(0, "/opt/trn_rl_repo")

import numpy as np
import ml_dtypes

import concourse.bass as bass
import concourse.bacc as bacc
import concourse.tile as tile
import concourse.mybir as mybir
from concourse import bass_utils

BF16 = mybir.dt.bfloat16
F32 = mybir.dt.float32
NPBF16 = ml_dtypes.bfloat16

B, T, D = 2, 2048, 1024
H, HD = 16, 64
NC = 8
HPC = H // NC          # heads per core = 2
TOK = B * T            # 4096
TPC = TOK // NC        # tokens per core after a2a = 512 (256 per batch)
NKB = T // 128         # 16 k-blocks per sequence
NQT = T // 512         # 4 q-tiles per sequence
KAUG = HD + 3          # 67: head_dim + 3 aug rows
MAXA = 8               # steep-head (slot A) k-block window per q-tile

_COMPILED = None


def _kept(hl, qt):
    """k-blocks computed for head-slot hl in q-tile qt (always even count,
    contiguous, ending at the diagonal block 4*qt+3)."""
    hi = 4 * qt + 4
    lo = max(0, hi - MAXA) if hl == 0 else 0
    return list(range(lo, hi))


def _build():
    nc = bacc.Bacc("TRN2", target_bir_lowering=False, debug=False, num_devices=NC)

    xT_d = nc.dram_tensor("xT", [D, TOK], BF16, kind="ExternalInput")
    wqkv_d = nc.dram_tensor("wqkv", [D, 384], BF16, kind="ExternalInput")
    wo_d = nc.dram_tensor("wo", [D, D], BF16, kind="ExternalInput")
    qaug_d = nc.dram_tensor("qaug", [HPC * 3, T], BF16, kind="ExternalInput")
    kaug_d = nc.dram_tensor("kaug", [3, T], BF16, kind="ExternalInput")
    kbias_d = nc.dram_tensor("kbias", [128, HPC], F32, kind="ExternalInput")
    cap_d = nc.dram_tensor("cap", [128, 128], F32, kind="ExternalInput")
    ind_d = nc.dram_tensor("ind", [1, 256], F32, kind="ExternalInput")
    out_d = nc.dram_tensor("out", [TPC, D], F32, kind="ExternalOutput")
    ccin = [
        nc.dram_tensor(f"ccin{b}", [NC * 128, TPC // B], BF16, kind="Internal")
        for b in range(B)
    ]
    ccout = [
        nc.dram_tensor(f"ccout{b}", [NC * 128, TPC // B], BF16, kind="Internal")
        for b in range(B)
    ]

    with tile.TileContext(nc) as tc:
        with (
            tc.tile_pool(name="const", bufs=1) as cpool,
            tc.tile_pool(name="work", bufs=1) as wpool,
            tc.tile_pool(name="pspair", bufs=2, space="PSUM") as pspair,
            tc.tile_pool(name="psot", bufs=2, space="PSUM") as psot,
            tc.tile_pool(name="ps", bufs=2, space="PSUM") as ps,
        ):
            # ---- sync queue: projection weights first, then xT chunks.
            # No DMA triggers ride the Scalar or Vector queues: a trigger
            # stalls its issuing compute engine when the HWDGE ring backs
            # up, and ScalarE owns the exp stream (the bottleneck).
            # one merged weight tile: chunk k occupies cols [384k, 384k+384)
            # as [wq_k | wk_k | wv_k]
            wqkv_t = cpool.tile([128, 8 * 384], BF16, name="wqkv_t", tag="wqkv_t")
            for k in range(8):
                nc.sync.dma_start(
                    wqkv_t[:, 384 * k : 384 * (k + 1)], wqkv_d[128 * k : 128 * (k + 1), :]
                )
            xt = [[None] * (TOK // 512) for _ in range(8)]
            for tc8 in range(TOK // 512):
                for k in range(8):
                    t_ = cpool.tile([128, 512], BF16, name=f"xt{k}_{tc8}", tag=f"xt{k}_{tc8}")
                    nc.sync.dma_start(t_[:], xT_d[128 * k : 128 * (k + 1), 512 * tc8 : 512 * (tc8 + 1)])
                    xt[k][tc8] = t_

            # ---- PE warm-up: dependency-free matmuls on scratch data so
            # the HAM clock gate reaches 8/8 before the real work arrives.
            warm_in = cpool.tile([128, 512], BF16, name="warm_in", tag="warm_in")
            nc.vector.memset(warm_in[:], 0.0)
            for _ in range(30):
                wps = psot.tile([128, 512], F32, name="wps", tag="otv")
                nc.tensor.matmul(wps[:], warm_in[:, 0:128], warm_in[:], start=True, stop=True)

            # ---- collective warm-up: a tiny AllToAll absorbs the
            # first-collective setup cost while the PE does projections.
            ccw_in = nc.dram_tensor("ccwin", [128, 16], BF16, kind="Internal")
            ccw_out = nc.dram_tensor("ccwout", [128, 16], BF16, kind="Internal")
            nc.gpsimd.dma_start(ccw_in[:], kaug_d[0:1, 0:2048].rearrange("a (p c) -> (a p) c", p=128))
            nc.gpsimd.collective_compute(
                "AllToAll",
                mybir.AluOpType.bypass,
                replica_groups=[list(range(NC))],
                ins=[ccw_in[:]],
                outs=[ccw_out[:]],
            )

            # ---- constants + aug rows on the (otherwise idle) GpSimd
            # queue, in consumption order.
            kbias_t = cpool.tile([128, HPC], F32, name="kbias_t", tag="kbias_t")
            nc.gpsimd.dma_start(kbias_t[:], kbias_d[:])
            cap_t = cpool.tile([128, 128], F32, name="cap_t", tag="cap_t")
            nc.gpsimd.dma_start(cap_t[:], cap_d[:])
            ind_t = cpool.tile([1, 256], F32, name="ind_t", tag="ind_t")
            nc.gpsimd.dma_start(ind_t[:], ind_d[:])

            qta = [[[None] * NQT for _ in range(HPC)] for _ in range(B)]
            kta = [[[None] * NQT for _ in range(HPC)] for _ in range(B)]
            for b in range(B):
                for hl in range(HPC):
                    for c in range(NQT):
                        q_ = cpool.tile([KAUG, 512], BF16, name=f"qta{b}{hl}{c}", tag=f"qta{b}{hl}{c}")
                        k_ = cpool.tile([KAUG, 512], BF16, name=f"kta{b}{hl}{c}", tag=f"kta{b}{hl}{c}")
                        qta[b][hl][c] = q_
                        kta[b][hl][c] = k_

            for b in range(B):
                for c in range(NQT):
                    for hl in range(HPC):
                        nc.gpsimd.dma_start(
                            qta[b][hl][c][64:67, :],
                            qaug_d[3 * hl : 3 * hl + 3, 512 * c : 512 * (c + 1)],
                        )
                        nc.gpsimd.dma_start(
                            kta[b][hl][c][64:67, :],
                            kaug_d[0:3, 512 * c : 512 * (c + 1)],
                        )
            # V: per (b, k-block): [128, 130]: 64 cols head A, ones col,
            # 64 cols head B, ones col.
            vt = [[None] * NKB for _ in range(B)]
            for b in range(B):
                for kb in range(NKB):
                    v_ = cpool.tile([128, 130], BF16, name=f"v{b}_{kb}", tag=f"v{b}_{kb}")
                    nc.vector.memset(v_.rearrange("p (a c) -> p a c", c=65)[:, :, 64], 1.0)
                    vt[b][kb] = v_

            # ---- phase 1: QKV projections (chunk-interleaved) ---------
            def qkv_chunk(tc8):
                b, cq = tc8 // NQT, tc8 % NQT
                for woff, dsts, eng in ((0, qta, "s"), (128, kta, "v")):
                    pp = ps.tile([128, 512], F32, name="pp", tag="mm512")
                    for k in range(8):
                        nc.tensor.matmul(
                            pp[:],
                            wqkv_t[:, 384 * k + woff : 384 * k + woff + 128],
                            xt[k][tc8][:],
                            start=(k == 0),
                            stop=(k == 7),
                        )
                    if eng == "s":
                        nc.scalar.copy(dsts[b][0][cq][0:64, :], pp[0:64, :])
                        nc.scalar.copy(dsts[b][1][cq][0:64, :], pp[64:128, :])
                    else:
                        nc.vector.tensor_copy(dsts[b][0][cq][0:64, :], pp[0:64, :])
                        nc.vector.tensor_copy(dsts[b][1][cq][0:64, :], pp[64:128, :])
                for j in range(4):
                    kb = 4 * cq + j
                    pv = ps.tile([128, 128], F32, name="pv", tag="mm512")
                    for k in range(8):
                        nc.tensor.matmul(
                            pv[:],
                            xt[k][tc8][:, 128 * j : 128 * (j + 1)],
                            wqkv_t[:, 384 * k + 256 : 384 * k + 384],
                            start=(k == 0),
                            stop=(k == 7),
                        )
                    nc.vector.tensor_copy(vt[b][kb][:, 0:64], pv[:, 0:64])
                    nc.vector.tensor_copy(vt[b][kb][:, 65:129], pv[:, 64:128])

            # ---- phase 2: attention for one (b, q-tile) ---------------
            def attn_group(b, qt):
                ots = []
                for hl in range(HPC):
                    ot = psot.tile([65, 512], F32, name="ot", tag="otv")
                    ots.append(ot)
                    kept = _kept(hl, qt)
                    pairs = [(kept[i], kept[i + 1]) for i in range(0, len(kept), 2)]
                    # In fully-diagonal pairs, put the larger column offset in
                    # slot 0: the single exp over [offs[0]:1024] then covers
                    # fewer dead columns. (Never the first pair, so the
                    # position-based start flag still covers all columns.)
                    pairs = [
                        (p[1], p[0]) if pi > 0 and p[0] >= 4 * qt else p
                        for pi, p in enumerate(pairs)
                    ]
                    pend = []
                    for pi in range(len(pairs) + 1):
                        if pi < len(pairs):
                            kb0, kb1 = pairs[pi]
                            offs = [max(0, 128 * (kb - 4 * qt)) for kb in (kb0, kb1)]
                            pr = pspair.tile([128, 1024], F32, name="pr", tag="pair")
                            for s, (kb, off) in enumerate(zip((kb0, kb1), offs)):
                                nc.tensor.matmul(
                                    pr[:, 512 * s + off : 512 * (s + 1)],
                                    kta[b][hl][kb // 4][:, 128 * (kb % 4) : 128 * (kb % 4 + 1)],
                                    qta[b][hl][qt][:, off:512],
                                    start=True,
                                    stop=True,
                                )
                                if kb >= 4 * qt:
                                    nc.vector.tensor_tensor(
                                        pr[:, 512 * s + off : 512 * s + off + 128],
                                        pr[:, 512 * s + off : 512 * s + off + 128],
                                        cap_t[:],
                                        mybir.AluOpType.min,
                                    )
                            ex = wpool.tile([128, 1024], BF16, name="ex", tag="ex", bufs=6)
                            nc.scalar.activation(
                                ex[:, offs[0] : 1024],
                                pr[:, offs[0] : 1024],
                                mybir.ActivationFunctionType.Exp,
                                bias=kbias_t[:, hl : hl + 1],
                                scale=0.125,
                            )
                            pend.append((pairs[pi], offs, ex))
                        if pi >= 1:
                            pj = pi - 1
                            (kb0, kb1), offs, ex = pend.pop(0)
                            for s, (kb, off) in enumerate(zip((kb0, kb1), offs)):
                                nc.tensor.matmul(
                                    ot[:, off:512],
                                    vt[b][kb][:, 65 * hl : 65 * hl + 65],
                                    ex[:, 512 * s + off : 512 * (s + 1)],
                                    start=(pj == 0 and s == 0),
                                    stop=(pj == len(pairs) - 1 and s == 1),
                                )
                # Copy OT out of PSUM immediately (one op per head, split
                # across ScalarE/VectorE) so the psot slots release for the
                # next group; the whole normalize chain then runs from SBUF
                # off the inter-group critical path. The very last group
                # instead normalizes straight from PSUM (nothing competes
                # for its psot slots) to shorten the path to the final
                # collective's doorbell.
                lean = b == B - 1 and qt == NQT - 1
                den2 = wpool.tile([1, 1024], F32, name="den2", tag="den2", bufs=2)
                if lean:
                    nc.vector.tensor_copy(den2[:, 0:512], ots[0][64:65, :])
                    nc.vector.tensor_copy(den2[:, 512:1024], ots[1][64:65, :])
                else:
                    otf0 = wpool.tile([65, 512], F32, name="otf0", tag="otf0", bufs=3)
                    otf1 = wpool.tile([128, 512], F32, name="otf1", tag="otf1", bufs=3)
                    denb = wpool.tile([1, 512], F32, name="denb", tag="denb", bufs=2)
                    nc.scalar.copy(otf0[:], ots[0][:])
                    nc.vector.tensor_copy(otf1[64:128, :], ots[1][0:64, :])
                    nc.vector.tensor_copy(denb[:], ots[1][64:65, :])
                    nc.vector.tensor_copy(den2[:, 0:512], otf0[64:65, :])
                    nc.vector.tensor_copy(den2[:, 512:1024], denb[:])
                bcs = wpool.tile([128, 1024], F32, name="bcs", tag="bcs", bufs=2)
                nc.gpsimd.partition_broadcast(bcs[:], den2[:])
                bci = wpool.tile([128, 1024], F32, name="bci", tag="bci", bufs=2)
                nc.vector.reciprocal_approx_fast(bci[:], bcs[:])
                otn = wpool.tile([128, 512], BF16, name="otn", tag="otn", bufs=4)
                if lean:
                    nc.vector.tensor_tensor(
                        otn[0:64, :], ots[0][0:64, :], bci[0:64, 0:512], mybir.AluOpType.mult
                    )
                    nc.vector.tensor_tensor(
                        otn[64:128, :], ots[1][0:64, :], bci[64:128, 512:1024], mybir.AluOpType.mult
                    )
                else:
                    nc.vector.tensor_tensor(
                        otn[0:64, :], otf0[0:64, :], bci[0:64, 0:512], mybir.AluOpType.mult
                    )
                    nc.vector.tensor_tensor(
                        otn[64:128, :], otf1[64:128, :], bci[64:128, 512:1024], mybir.AluOpType.mult
                    )
                # two destination blocks of 256 tokens each
                for half in range(2):
                    blk = 2 * qt + half
                    nc.sync.dma_start(
                        ccin[b][128 * blk : 128 * (blk + 1), :],
                        otn[:, 256 * half : 256 * (half + 1)],
                    )

            # ---- phase 4: output projection for one batch -------------
            at = [[None] * 8 for _ in range(B)]

            def yrecv(b):
                for k in range(8):
                    a_ = cpool.tile([128, TPC // B], BF16, name=f"at{b}_{k}", tag=f"at{b}_{k}")
                    nc.sync.dma_start(a_[:], ccout[b][128 * k : 128 * (k + 1), :])
                    at[b][k] = a_

            def ypiece(b, tb, n):
                yp = ps.tile([128, 512], F32, name="yp", tag="mm512")
                for k in range(8):
                    nc.tensor.matmul(
                        yp[:],
                        at[b][k][:, 128 * tb : 128 * (tb + 1)],
                        wo_t[:, D * k + 512 * n : D * k + 512 * (n + 1)],
                        start=(k == 0),
                        stop=(k == 7),
                    )
                ys = wpool.tile([128, 512], F32, name="ys", tag="ys", bufs=2)
                nc.vector.tensor_copy(ys[:], yp[:])
                nc.sync.dma_start(
                    out_d[256 * b + 128 * tb : 256 * b + 128 * (tb + 1), 512 * n : 512 * (n + 1)],
                    ys[:],
                )

            # ---- schedule -------------------------------------------
            for qt in range(NQT):
                qkv_chunk(qt)
                attn_group(0, qt)
            # wo arrives during attention on the sync queue
            wo_t = cpool.tile([128, 8 * D], BF16, name="wo_t", tag="wo_t")
            for k in range(8):
                nc.sync.dma_start(wo_t[:, D * k : D * (k + 1)], wo_d[128 * k : 128 * (k + 1), :])
            nc.gpsimd.collective_compute(
                "AllToAll",
                mybir.AluOpType.bypass,
                replica_groups=[list(range(NC))],
                ins=[ccin[0][:]],
                outs=[ccout[0][:]],
            )
            yrecv(0)
            for qt in range(NQT):
                qkv_chunk(NQT + qt)
                attn_group(1, qt)
            for tb in range(2):
                for n in range(D // 512):
                    ypiece(0, tb, n)
            nc.gpsimd.collective_compute(
                "AllToAll",
                mybir.AluOpType.bypass,
                replica_groups=[list(range(NC))],
                ins=[ccin[1][:]],
                outs=[ccout[1][:]],
            )
            yrecv(1)
            for tb in range(2):
                for n in range(D // 512):
                    ypiece(1, tb, n)

    nc.compile()
    return nc


def _host_inputs(x, Wq, Wk, Wv, Wo):
    x = np.asarray(x, dtype=np.float32)
    Wq, Wk, Wv, Wo = (np.asarray(w, dtype=np.float32) for w in (Wq, Wk, Wv, Wo))
    toks = x.reshape(TOK, D)
    xT = np.ascontiguousarray(toks.T).astype(NPBF16)
    base = 2.0 ** (-8.0 / H)

    cap = np.where(
        np.arange(128)[:, None] <= np.arange(128)[None, :], 3.0e38, -1.0e9
    ).astype(np.float32)
    ind = np.zeros((1, 256), dtype=np.float32)
    ind[0, 0:64] = 1      # head-A indicator: bc rows 0:64 get denA
    ind[0, 192:256] = 1   # head-B indicator: bc rows 64:128 get denB
    pos = np.arange(T, dtype=np.float32)
    pos_bf = pos.astype(NPBF16).astype(np.float32)
    kbrow = np.floor(pos / 128.0).astype(NPBF16)  # k-block index, exact
    ones_row = np.ones(T, dtype=NPBF16)
    kaug = np.stack([ones_row, kbrow, kbrow])  # rows 64..66 of K'

    in_maps = []
    for c in range(NC):
        heads = [c, 8 + c]  # steep slot A, shallow slot B
        rows = np.concatenate([np.arange(64 * g, 64 * (g + 1)) for g in heads])
        qaug = np.zeros((HPC * 3, T), dtype=NPBF16)
        kbias = np.zeros((128, HPC), dtype=np.float32)
        for hl, g in enumerate(heads):
            slope = float(base ** (g + 1))
            qaug[3 * hl + 0] = (-8.0 * slope * pos_bf).astype(NPBF16)
            cc = NPBF16(1024.0 * slope)
            dr = NPBF16(1024.0 * slope - float(cc))
            qaug[3 * hl + 1] = cc
            qaug[3 * hl + 2] = dr
            kbias[:, hl] = slope * np.arange(128)
        in_maps.append(
            {
                "xT": xT,
                "wqkv": np.ascontiguousarray(
                    np.concatenate(
                        [Wq[rows, :].T, Wk[rows, :].T, Wv[rows, :].T], axis=1
                    )
                ).astype(NPBF16),
                "wo": None,  # filled below (same for all cores)
                "qaug": qaug,
                "kaug": kaug,
                "kbias": kbias,
                "cap": cap,
                "ind": ind,
            }
        )
    # Wo rows permuted to match the concat order the a2a produces:
    # source core p contributes [head p dims ; head 8+p dims].
    perm = np.concatenate(
        [
            np.concatenate(
                [np.arange(64 * p, 64 * (p + 1)), np.arange(64 * (8 + p), 64 * (9 + p))]
            )
            for p in range(NC)
        ]
    )
    wo_t = np.ascontiguousarray(Wo.T[perm, :]).astype(NPBF16)
    for m in in_maps:
        m["wo"] = wo_t
    return in_maps


def get_compiled():
    global _COMPILED
    if _COMPILED is None:
        _COMPILED = _build()
    return _COMPILED


def run(x, Wq, Wk, Wv, Wo, trace=False, **trace_kwargs):
    nc = get_compiled()
    in_maps = _host_inputs(x, Wq, Wk, Wv, Wo)
    res = bass_utils.run_bass_kernel_spmd(
        nc, in_maps, core_ids=list(range(NC)), trace=trace, **trace_kwargs
    )
    full = np.empty((TOK, D), dtype=np.float32)
    half = TPC // B  # 256
    for c in range(NC):
        o = res.results[c]["out"]
        full[half * c : half * (c + 1), :] = o[0:half]
        full[T + half * c : T + half * (c + 1), :] = o[half : 2 * half]
    return full.reshape(B, T, D), res


def kernel(x, Wq, Wk, Wv, Wo):
    out, _ = run(x, Wq, Wk, Wv, Wo)
    return out
